# revision 7
# baseline (speedup 1.0000x reference)
"""Trainium2 Bass kernel for nn_BGAN (GNN message passing), 8 NeuronCores.

Node-sharded SPMD with replicated weights:
  A. z-phase: zx = h_tile @ [W_fc.T | v | W_gc] (bf16 PE, f32 PSUM) where
     v = W_fc.T @ [a_src | w_row0 | w_row1 | a_dst] is host-folded.  Each
     node's 272B table row = z (fp8e4, 256B) + {e_src, zw0, zw1, hw} f32.
     hw = (h@W_gc)*n_src with n_src = rsqrt(out-degree) host-precomputed
     from `neighbors` (index-only preprocessing, like widx).
  B. One AllGather of the 33MiB row table.
  C. mailbox: ONE batched indirect gather per 512-node chunk (5120 rows);
     attention softmax; row conv from gathered scalars; col conv via per-k
     diagonal matmuls accumulated in PSUM; updatefeat matmul.
  D. GraphConv agg -> group softmax weights -> weighted mean folded into
     the final matmul -> AllGather partials -> classifier.

kernel(**inputs): FULL numpy inputs -> FULL [1, C] output.
"""
import sys
import types

import numpy as np

sys.path.insert(0, "/opt/trn_rl_repo")

import concourse.bass as bass
import concourse.bacc as bacc
import concourse.mybir as mybir
import concourse.tile as tile
from concourse import bass_utils
from concourse.bass import broadcast_tensor_aps
from concourse.masks import make_identity
from concourse.tile import add_dep_helper

P = 128
D = 256
K = 10
C_CLS = 40
NCORES = 8
EPS = 1e-5

NODE_F32 = 68            # 272B node row: 256 fp8e4 z + 4 f32 scalars
SC_ESRC = 64
SC_ZW0 = 65
SC_ZW1 = 66
SC_HW = 67

F32 = mybir.dt.float32
BF16 = mybir.dt.bfloat16
FP8 = mybir.dt.float8e4
I32 = mybir.dt.int32
AF = mybir.ActivationFunctionType
ALU = mybir.AluOpType
AX = mybir.AxisListType


def _ntff_hook():
    try:
        import antenv
        from trn_agent_boot.trn_boot import _ntff_profile_via_ctypes
        mod = types.ModuleType("antenv.axon_hooks")
        _state = {"hook": None}
        mod.set_axon_ntff_profile_hook = lambda h: _state.update(hook=h)
        mod.get_axon_ntff_profile_hook = lambda: _state["hook"]
        sys.modules["antenv.axon_hooks"] = mod
        antenv.axon_hooks = mod
        mod.set_axon_ntff_profile_hook(
            _ntff_profile_via_ctypes("/opt/axon/libaxon_pjrt.so"))
    except Exception:
        pass


def bc(a, b):
    """broadcast b against a, return broadcasted b."""
    _, b2 = broadcast_tensor_aps(a, b)
    return b2


def build(n_nodes, scal, shared_zp=True):
    NLOC = n_nodes // NCORES
    NT = NLOC // P
    NCH = NLOC // 512
    NG = NLOC // 256

    nc = bacc.Bacc("TRN2", num_devices=NCORES, dynamic_dma_scratch_size=32768)
    rg = [list(range(NCORES))]

    h_in = nc.dram_tensor("h", [NLOC, D], F32, kind="ExternalInput")
    hti = nc.dram_tensor("hti", [P, NT, 2, P], BF16, kind="ExternalInput")
    rext_i = nc.dram_tensor("rext", [D, 261], BF16, kind="ExternalInput")
    nsrc_i = nc.dram_tensor("nsrc", [P, NT], F32, kind="ExternalInput")
    wcol_i = nc.dram_tensor("wcol", [1, K], F32, kind="ExternalInput")
    lw = nc.dram_tensor("lw", [K - 1 + D, D], BF16, kind="ExternalInput")
    wcls = nc.dram_tensor("wcls", [D, C_CLS], F32, kind="ExternalInput")
    bcls = nc.dram_tensor("bcls", [1, C_CLS], F32, kind="ExternalInput")
    widx = nc.dram_tensor("widx", [NCH, P, 40], I32, kind="ExternalInput")

    out_t = nc.dram_tensor("out", [1, C_CLS], F32, kind="ExternalOutput")

    zp_sh = nc.dram_tensor("zp_sh", [NLOC, NODE_F32], F32, kind="Internal")
    zp_full = nc.dram_tensor(
        "zp_full", [n_nodes, NODE_F32], F32, kind="Internal",
        addr_space="Shared" if shared_zp else "Local")
    hgp_part = nc.dram_tensor("hgp_part", [1, D], F32, kind="Internal")
    hgp_full = nc.dram_tensor("hgp_full", [NCORES, D], F32, kind="Internal",
                              addr_space="Shared")

    with tile.TileContext(nc) as tc:
        with tc.tile_pool(name="cst", bufs=1) as cst, \
             tc.tile_pool(name="sb", bufs=2) as sb, \
             tc.tile_pool(name="res", bufs=1) as res:

            ident = cst.tile([P, P], F32)
            make_identity(nc, ident[:, :])
            identb = cst.tile([P, P], BF16)
            nc.vector.tensor_copy(out=identb[:, :], in_=ident[:, :])
            identq = cst.tile([P, P], FP8)
            nc.vector.tensor_copy(out=identq[:, :], in_=ident[:, :])
            onesf = cst.tile([P, 1], F32)
            nc.vector.memset(onesf[:, :], 1.0)

            rext = cst.tile([P, 2, 261], BF16)
            nc.sync.dma_start(out=rext[:, 0, :], in_=rext_i[0:P, :])
            nc.sync.dma_start(out=rext[:, 1, :], in_=rext_i[P:D, :])
            nsrc_sb = cst.tile([P, NT], F32)
            nc.sync.dma_start(out=nsrc_sb[:, :], in_=nsrc_i[:, :])
            wcol_rep = cst.tile([P, K], F32)
            nc.sync.dma_start(out=wcol_rep[:, :],
                              in_=wcol_i[0:1, :].to_broadcast([P, K]))
            lw_sb = cst.tile([P, 3, D], BF16)
            nc.vector.memset(lw_sb[:, 0, :], 0.0)
            nc.sync.dma_start(out=lw_sb[0:K - 1, 0, :], in_=lw[0:K - 1, :])
            nc.sync.dma_start(out=lw_sb[:, 1, :], in_=lw[K - 1:K - 1 + P, :])
            nc.sync.dma_start(out=lw_sb[:, 2, :], in_=lw[K - 1 + P:K - 1 + D, :])

            edst_res = res.tile([P, NT], F32)
            ufr = res.tile([P, NT * D], BF16)
            hgs_acc = res.tile([1, D], F32)
            nc.vector.memset(hgs_acc[:, :], 0.0)
            widx_sb = res.tile([P, NCH, 40], I32)
            nc.sync.dma_start(out=widx_sb[:, :, :],
                              in_=widx.rearrange("c p k -> p c k"))

            # ================= phase A: z rows =================
            with tc.tile_pool(name="psa", bufs=2, space="PSUM") as psa:
                zp_wr = []
                GSZ = 8 if NT % 8 == 0 else 4
                for b in range(NT // GSZ):
                    hT4 = sb.tile([P, GSZ, 2, P], BF16, tag="hT4")
                    nc.sync.dma_start(out=hT4[:, :, :, :],
                                      in_=hti[:, GSZ * b:GSZ * b + GSZ, :, :])
                    stg4 = sb.tile([P, GSZ, NODE_F32], F32, tag="stg4")
                    for i in range(GSZ):
                        t = GSZ * b + i
                        zx = psa.tile([P, 261], F32, space="PSUM", tag="zx")
                        nc.tensor.matmul(out=zx[:, :], lhsT=hT4[:, i, 0, :],
                                         rhs=rext[:, 0, :], start=True, stop=False)
                        nc.tensor.matmul(out=zx[:, :], lhsT=hT4[:, i, 1, :],
                                         rhs=rext[:, 1, :], start=False, stop=True)
                        stgq = stg4[:, i, :].bitcast(FP8)
                        nc.scalar.activation(out=stgq[:, 0:D], in_=zx[:, 0:D],
                                             func=AF.Copy)
                        nc.vector.tensor_copy(out=stg4[:, i, SC_ESRC:SC_ESRC + 3],
                                              in_=zx[:, 256:259])
                        nc.vector.tensor_copy(out=edst_res[:, t:t + 1],
                                              in_=zx[:, 259:260])
                        nc.vector.tensor_tensor(out=stg4[:, i, SC_HW:SC_HW + 1],
                                                in0=zx[:, 260:261],
                                                in1=nsrc_sb[:, t:t + 1], op=ALU.mult)
                    w1 = nc.sync.dma_start(
                        out=zp_sh[GSZ * b * P:(GSZ * b + GSZ) * P, :].rearrange(
                            "(a p) f -> p a f", p=P),
                        in_=stg4[:, :, :])
                    zp_wr.append(w1)

            cc_zp = nc.gpsimd.collective_compute(
                "AllGather", ALU.bypass, ins=[zp_sh[:, :]], outs=[zp_full[:, :]],
                replica_groups=rg)
            for w in zp_wr:
                add_dep_helper(cc_zp.ins, w.ins, True, "zp AG after writes")

            # ================= phase C: mailbox =================
            with tc.tile_pool(name="mailp", bufs=6) as mailp, \
                 tc.tile_pool(name="psm", bufs=2, space="PSUM") as psm, \
                 tc.tile_pool(name="psx", bufs=1, space="PSUM") as psx:
                for chh in range(NCH):
                    mail = mailp.tile([P, 40, NODE_F32], F32, tag="mail")
                    for cc_ in range(40):
                        g = nc.gpsimd.indirect_dma_start(
                            out=mail[:, cc_, :],
                            out_offset=None, in_=zp_full[:, :],
                            in_offset=bass.IndirectOffsetOnAxis(
                                ap=widx_sb[:, chh, cc_:cc_ + 1], axis=0))
                        if cc_ == 0:
                            add_dep_helper(g.ins, cc_zp.ins, True,
                                           "gather after zp AG")
                    h_c = sb.tile([P, 4, D], F32, tag="h_c")
                    nc.sync.dma_start(
                        out=h_c[:, :, :],
                        in_=h_in[chh * 512:(chh + 1) * 512, :].rearrange(
                            "(a p) f -> p a f", p=P))

                    esr = mail[:, :, SC_ESRC]
                    zw0 = mail[:, :, SC_ZW0]
                    zw1 = mail[:, :, SC_ZW1]
                    hwg = mail[:, :, SC_HW]

                    ee = sb.tile([P, 40], F32, tag="ee")
                    e3 = ee[:, :].rearrange("p (k j) -> p k j", k=K)
                    ed3 = edst_res[:, chh * 4:(chh + 1) * 4].rearrange(
                        "p (o j) -> p o j", o=1)
                    nc.vector.tensor_tensor(
                        out=e3, in0=esr.rearrange("p (k j) -> p k j", k=K),
                        in1=bc(e3, ed3), op=ALU.add)
                    eesc = sb.tile([P, 40], F32, tag="eesc")
                    nc.vector.tensor_scalar(out=eesc[:, :], in0=ee[:, :],
                                            scalar1=0.01, scalar2=None, op0=ALU.mult)
                    nc.vector.tensor_tensor(out=ee[:, :], in0=ee[:, :],
                                            in1=eesc[:, :], op=ALU.max)
                    emax = sb.tile([P, 4], F32, tag="emax")
                    nc.vector.tensor_reduce(
                        out=emax[:, :], in_=ee[:, :].rearrange("p (k j) -> p j k", k=K),
                        axis=AX.X, op=ALU.max)
                    es = sb.tile([P, 40], F32, tag="es")
                    es3 = es[:, :].rearrange("p (k j) -> p k j", k=K)
                    nc.vector.tensor_tensor(
                        out=es3, in0=e3,
                        in1=bc(es3, emax[:, :].rearrange("p (o j) -> p o j", o=1)),
                        op=ALU.subtract)
                    ex = sb.tile([P, 40], F32, tag="ex")
                    nc.scalar.activation(out=ex[:, :], in_=es[:, :], func=AF.Exp)
                    esum = sb.tile([P, 4], F32, tag="esum")
                    nc.vector.tensor_reduce(
                        out=esum[:, :], in_=ex[:, :].rearrange("p (k j) -> p j k", k=K),
                        axis=AX.X, op=ALU.add)
                    erec = sb.tile([P, 4], F32, tag="erec")
                    nc.vector.reciprocal(out=erec[:, :], in_=esum[:, :])
                    alp = sb.tile([P, 40], F32, tag="alp")
                    al3 = alp[:, :].rearrange("p (k j) -> p k j", k=K)
                    nc.vector.tensor_tensor(
                        out=al3, in0=ex[:, :].rearrange("p (k j) -> p k j", k=K),
                        in1=bc(al3, erec[:, :].rearrange("p (o j) -> p o j", o=1)),
                        op=ALU.mult)

                    bet = sb.tile([P, 40], F32, tag="bet")
                    b3 = bet[:, :].rearrange("p (k j) -> p k j", k=K)
                    nc.vector.tensor_tensor(
                        out=b3, in0=al3,
                        in1=bc(b3, wcol_rep[:, :].rearrange("p (k o) -> p k o", o=1)),
                        op=ALU.mult)

                    agt = sb.tile([P, 4], F32, tag="agt")
                    nc.vector.tensor_reduce(
                        out=agt[:, :], in_=hwg.rearrange("p (k j) -> p j k", k=K),
                        axis=AX.X, op=ALU.add)

                    r0 = sb.tile([P, 40], F32, tag="r0")
                    r1_ = sb.tile([P, 40], F32, tag="r1_")
                    nc.vector.tensor_tensor(out=r0[:, :], in0=alp[:, :], in1=zw0,
                                            op=ALU.mult)
                    nc.vector.tensor_tensor(out=r1_[:, :], in0=alp[:, :], in1=zw1,
                                            op=ALU.mult)
                    rowp = sb.tile([P, 4, 16], F32, tag="rowp")
                    nc.vector.memset(rowp[:, :, K - 1:16], 0.0)
                    nc.vector.tensor_tensor(
                        out=rowp[:, :, 0:K - 1].rearrange("p j k -> p k j"),
                        in0=r0[:, :].rearrange("p (k j) -> p k j", k=K)[:, 0:K - 1, :],
                        in1=r1_[:, :].rearrange("p (k j) -> p k j", k=K)[:, 1:K, :],
                        op=ALU.add)
                    nc.scalar.activation(out=rowp[:, :, 0:K - 1],
                                         in_=rowp[:, :, 0:K - 1], func=AF.Relu,
                                         bias=scal["bias_row"], scale=scal["s_row"])

                    mailq = mail[:, :, :].bitcast(FP8)
                    for jj in range(4):
                        t = chh * 4 + jj
                        colp = psm.tile([P, D], F32, space="PSUM", tag="colp")
                        for k in range(K):
                            cidx = k * 4 + jj
                            dg_ = sb.tile([P, P], FP8, tag="diag")
                            nc.vector.tensor_scalar(
                                out=dg_[:, :], in0=identq[:, :],
                                scalar1=bet[:, cidx:cidx + 1], scalar2=None,
                                op0=ALU.mult)
                            nc.tensor.matmul(
                                out=colp[:, :], lhsT=dg_[:, :],
                                rhs=mailq[:, cidx, 0:D],
                                start=(k == 0), stop=(k == K - 1))
                        colr = sb.tile([P, D], BF16, tag="colr")
                        nc.scalar.activation(out=colr[:, :], in_=colp[:, :],
                                             func=AF.Relu, bias=scal["bias_col"],
                                             scale=scal["s_col"])
                        ctp = psx.tile([P, 2, P], BF16, space="PSUM", tag="ctp")
                        nc.tensor.transpose(out=ctp[:, 0, :], in_=colr[:, 0:P],
                                            identity=identb[:, :])
                        nc.tensor.transpose(out=ctp[:, 1, :], in_=colr[:, P:D],
                                            identity=identb[:, :])
                        colT = sb.tile([P, 2, P], BF16, tag="colT")
                        nc.scalar.copy(out=colT[:, 0, :], in_=ctp[:, 0, :])
                        nc.scalar.copy(out=colT[:, 1, :], in_=ctp[:, 1, :])
                        rtp = psx.tile([16, P], F32, space="PSUM", tag="rtp")
                        nc.tensor.transpose(out=rtp[:, :], in_=rowp[:, jj, :],
                                            identity=ident[:, :])
                        rowT = sb.tile([16, P], BF16, tag="rowT")
                        nc.scalar.copy(out=rowT[:, :], in_=rtp[:, :])
                        ufp = psm.tile([P, D], F32, space="PSUM", tag="ufp")
                        nc.tensor.matmul(out=ufp[:, :], lhsT=rowT[0:K - 1, :],
                                         rhs=lw_sb[0:K - 1, 0, :], start=True, stop=False)
                        nc.tensor.matmul(out=ufp[:, :], lhsT=colT[:, 0, :],
                                         rhs=lw_sb[:, 1, :], start=False, stop=False)
                        nc.tensor.matmul(out=ufp[:, :], lhsT=colT[:, 1, :],
                                         rhs=lw_sb[:, 2, :], start=False, stop=True)
                        ufs = sb.tile([P, D], F32, tag="ufs")
                        nc.vector.tensor_tensor(out=ufs[:, :], in0=ufp[:, :],
                                                in1=h_c[:, jj, :], op=ALU.add)
                        nc.scalar.activation(out=ufr[:, t * D:(t + 1) * D],
                                             in_=ufs[:, :], func=AF.Relu)

                    # group softmax weights (groups of 256 nodes = 2 tiles)
                    # and weighted-mean partials, fused into the chunk
                    eagg = sb.tile([P, 4], BF16, tag="eagg")
                    nc.scalar.activation(out=eagg[:, :], in_=agt[:, :],
                                         func=AF.Exp, bias=scal["b_gc"],
                                         scale=scal["n_dst"])
                    egv = eagg[:, :].rearrange("p (g a) -> p g a", a=2)
                    eg2 = sb.tile([P, 2], F32, tag="eg2")
                    nc.vector.tensor_tensor(out=eg2[:, :], in0=egv[:, :, 0],
                                            in1=egv[:, :, 1], op=ALU.add)
                    zs = psx.tile([1, 2], F32, space="PSUM", tag="zs")
                    nc.tensor.matmul(out=zs[:, :], lhsT=onesf[:, :],
                                     rhs=eg2[:, :], start=True, stop=True)
                    zsum = sb.tile([1, 2], F32, tag="zsum")
                    nc.vector.tensor_copy(out=zsum[:, :], in_=zs[:, :])
                    rz = sb.tile([1, 2], F32, tag="rz")
                    nc.vector.reciprocal(out=rz[:, :], in_=zsum[:, :])
                    for a in range(2):
                        hgq = psx.tile([1, D], F32, space="PSUM", tag="hgq")
                        for j2 in range(2):
                            jj = 2 * a + j2
                            t = chh * 4 + jj
                            nc.tensor.matmul(
                                out=hgq[:, :], lhsT=eagg[:, jj:jj + 1],
                                rhs=ufr[:, t * D:(t + 1) * D],
                                start=(j2 == 0), stop=(j2 == 1))
                        hsc = sb.tile([1, D], F32, tag="hsc")
                        nc.vector.tensor_scalar(
                            out=hsc[:, :], in0=hgq[:, :],
                            scalar1=rz[:, a:a + 1], scalar2=scal["inv_n"],
                            op0=ALU.mult, op1=ALU.mult)
                        nc.vector.tensor_tensor(
                            out=hgs_acc[:, :], in0=hgs_acc[:, :],
                            in1=hsc[:, :], op=ALU.add)

            # ================= phase D: final reduction =================
            with tc.tile_pool(name="pse", bufs=1, space="PSUM") as pse:
                hw3 = nc.sync.dma_start(out=hgp_part[:, :], in_=hgs_acc[:, :])
                cc_hg = nc.gpsimd.collective_compute(
                    "AllGather", ALU.bypass, ins=[hgp_part[:, :]], outs=[hgp_full[:, :]],
                    replica_groups=rg)
                add_dep_helper(cc_hg.ins, hw3.ins, True, "hg AG after write")
                hgf = sb.tile([P, 2, NCORES], F32, tag="hgf")
                for m in range(2):
                    hr = nc.sync.dma_start(
                        out=hgf[:, m, :],
                        in_=hgp_full[:, m * P:(m + 1) * P].rearrange("c p -> p c"))
                    add_dep_helper(hr.ins, cc_hg.ins, True, "hg read after AG")
                hg = sb.tile([P, 2], F32, tag="hg")
                nc.vector.tensor_reduce(
                    out=hg[:, :], in_=hgf[:, :, :], axis=AX.X, op=ALU.add)
                wcls_sb = sb.tile([P, 2, C_CLS], F32, tag="wcls_sb")
                nc.sync.dma_start(out=wcls_sb[:, 0, :], in_=wcls[0:P, :])
                nc.sync.dma_start(out=wcls_sb[:, 1, :], in_=wcls[P:D, :])
                outp = pse.tile([1, C_CLS], F32, space="PSUM", tag="outp")
                for m in range(2):
                    nc.tensor.matmul(out=outp[:, :], lhsT=hg[:, m:m + 1],
                                     rhs=wcls_sb[:, m, :], start=(m == 0), stop=(m == 1))
                bcl = sb.tile([1, C_CLS], F32, tag="bcl")
                nc.sync.dma_start(out=bcl[:, :], in_=bcls[:, :])
                oo = sb.tile([1, C_CLS], F32, tag="oo")
                nc.vector.tensor_tensor(out=oo[:, :], in0=outp[:, :], in1=bcl[:, :],
                                        op=ALU.add)
                nc.sync.dma_start(out=out_t[:, :], in_=oo[:, :])

    return nc


def prep_inputs(h, neighbors, W_fc, a_attn, w_row, b_row, g_row, be_row,
                w_col, b_col, g_col, be_col, localw, W_gc, b_gc, W_cls, b_cls):
    import ml_dtypes
    h = np.asarray(h)
    n_nodes = h.shape[0]
    NLOC = n_nodes // NCORES
    NT = NLOC // P
    NCH = NLOC // 512
    nb = np.asarray(neighbors).astype(np.int64)
    a_attn = np.asarray(a_attn)
    w_row = np.asarray(w_row)

    s_row = float(np.float32(np.asarray(g_row)[0]) / np.sqrt(np.float32(1.0 + EPS)))
    s_col = float(np.float32(np.asarray(g_col)[0]) / np.sqrt(np.float32(1.0 + EPS)))
    scal = dict(
        s_row=s_row,
        bias_row=float(np.float32(np.asarray(b_row)[0]) * np.float32(s_row)
                       + np.float32(np.asarray(be_row)[0])),
        s_col=s_col,
        bias_col=float(np.float32(np.asarray(b_col)[0]) * np.float32(s_col)
                       + np.float32(np.asarray(be_col)[0])),
        n_dst=float(1.0 / np.sqrt(np.float32(K))),
        b_gc=float(np.asarray(b_gc)[0]),
        inv_n=float(np.float32(1.0) / np.float32(n_nodes)),
    )

    deg = np.bincount(nb.ravel(), minlength=n_nodes).astype(np.float64)
    n_src_all = np.where(deg > 0, 1.0 / np.sqrt(np.maximum(deg, 1.0)), 0.0)
    n_src_all = n_src_all.astype(np.float32)

    wfct = np.asarray(W_fc).T.astype(np.float32)
    awp = np.stack([a_attn[:D], w_row[0], w_row[1], a_attn[D:]],
                   axis=1).astype(np.float32)
    v = wfct @ awp                                          # [D, 4]
    rext = np.concatenate(
        [wfct, v, np.asarray(W_gc).astype(np.float32).reshape(D, 1)],
        axis=1).astype(ml_dtypes.bfloat16)                  # [D, 261]

    common = {
        "rext": np.ascontiguousarray(rext),
        "wcol": np.asarray(w_col).astype(np.float32).reshape(1, K),
        "lw": np.asarray(localw).astype(ml_dtypes.bfloat16),
        "wcls": np.asarray(W_cls).astype(np.float32),
        "bcls": np.asarray(b_cls).astype(np.float32).reshape(1, C_CLS),
    }

    in_maps = []
    for c in range(NCORES):
        nbl = nb[c * NLOC:(c + 1) * NLOC]
        widx = np.ascontiguousarray(
            nbl.reshape(NCH, 4, P, K).transpose(0, 2, 3, 1)
            .reshape(NCH, P, 40).astype(np.int32))
        h_loc = h[c * NLOC:(c + 1) * NLOC].astype(np.float32)
        hti = np.ascontiguousarray(
            h_loc.reshape(NT, P, 2, P).transpose(3, 0, 2, 1)
            .astype(ml_dtypes.bfloat16))
        nsrc = np.ascontiguousarray(
            n_src_all[c * NLOC:(c + 1) * NLOC].reshape(NT, P).T)
        m = {
            "h": np.ascontiguousarray(h_loc),
            "hti": hti,
            "nsrc": nsrc,
            "widx": widx,
        }
        m.update(common)
        in_maps.append(m)
    return in_maps, scal, n_nodes


_CACHE = {}


def run(inputs, trace=False):
    _ntff_hook()
    in_maps, scal, n_nodes = prep_inputs(**inputs)
    key = (n_nodes, tuple(sorted(scal.items())))
    if key not in _CACHE:
        nc = build(n_nodes, scal)
        nc.finalize()
        _CACHE[key] = nc
    nc = _CACHE[key]
    return bass_utils.run_bass_kernel_spmd(
        nc, in_maps, core_ids=list(range(NCORES)), trace=trace)


def kernel(**inputs):
    res = run(inputs, trace=False)
    return np.asarray(res.results[0]["out"], dtype=np.float32)


# revision 8
# speedup vs baseline: 1.1837x; 1.1837x over previous
"""Trainium2 Bass kernel for nn_BGAN (GNN message passing), 8 NeuronCores.

Node-sharded SPMD with replicated weights:
  A. z-phase: zx = h_tile @ [W_fc.T | v | W_gc] (bf16 PE, f32 PSUM) where
     v = W_fc.T @ [a_src | w_row0 | w_row1 | a_dst] is host-folded.  Each
     node's 272B table row = z (fp8e4, 256B) + {e_src, zw0, zw1, hw} f32.
     hw = (h@W_gc)*n_src with n_src = rsqrt(out-degree) host-precomputed
     from `neighbors` (index-only preprocessing, like widx).
  B. One AllGather of the 33MiB row table.
  C. mailbox: ONE batched indirect gather per 512-node chunk (5120 rows);
     attention softmax; row conv from gathered scalars; col conv via per-k
     diagonal matmuls accumulated in PSUM; updatefeat matmul.
  D. GraphConv agg -> group softmax weights -> weighted mean folded into
     the final matmul -> AllGather partials -> classifier.

kernel(**inputs): FULL numpy inputs -> FULL [1, C] output.
"""
import sys
import types

import numpy as np

sys.path.insert(0, "/opt/trn_rl_repo")

import concourse.bass as bass
import concourse.bacc as bacc
import concourse.mybir as mybir
import concourse.tile as tile
from concourse import bass_utils
from concourse.bass import broadcast_tensor_aps
from concourse.masks import make_identity
from concourse.tile import add_dep_helper

P = 128
D = 256
K = 10
C_CLS = 40
NCORES = 8
EPS = 1e-5

NODE_F32 = 68            # 272B node row: 256 fp8e4 z + 4 f32 scalars
SC_ESRC = 64
SC_ZW0 = 65
SC_ZW1 = 66
SC_HW = 67

F32 = mybir.dt.float32
BF16 = mybir.dt.bfloat16
FP8 = mybir.dt.float8e4
I32 = mybir.dt.int32
AF = mybir.ActivationFunctionType
ALU = mybir.AluOpType
AX = mybir.AxisListType


def _ntff_hook():
    try:
        import antenv
        from trn_agent_boot.trn_boot import _ntff_profile_via_ctypes
        mod = types.ModuleType("antenv.axon_hooks")
        _state = {"hook": None}
        mod.set_axon_ntff_profile_hook = lambda h: _state.update(hook=h)
        mod.get_axon_ntff_profile_hook = lambda: _state["hook"]
        sys.modules["antenv.axon_hooks"] = mod
        antenv.axon_hooks = mod
        mod.set_axon_ntff_profile_hook(
            _ntff_profile_via_ctypes("/opt/axon/libaxon_pjrt.so"))
    except Exception:
        pass


def bc(a, b):
    """broadcast b against a, return broadcasted b."""
    _, b2 = broadcast_tensor_aps(a, b)
    return b2


def build(n_nodes, scal, shared_zp=True):
    NLOC = n_nodes // NCORES
    NT = NLOC // P
    NCH = NLOC // 512
    NG = NLOC // 256

    nc = bacc.Bacc("TRN2", num_devices=NCORES, dynamic_dma_scratch_size=32768)
    rg = [list(range(NCORES))]

    h_in = nc.dram_tensor("h", [NLOC, D], F32, kind="ExternalInput")
    hti = nc.dram_tensor("hti", [P, NT, 2, P], BF16, kind="ExternalInput")
    rext_i = nc.dram_tensor("rext", [D, 261], BF16, kind="ExternalInput")
    nsrc_i = nc.dram_tensor("nsrc", [P, NT], F32, kind="ExternalInput")
    wcol_i = nc.dram_tensor("wcol", [1, K], F32, kind="ExternalInput")
    lw = nc.dram_tensor("lw", [K - 1 + D, D], BF16, kind="ExternalInput")
    wcls = nc.dram_tensor("wcls", [D, C_CLS], F32, kind="ExternalInput")
    bcls = nc.dram_tensor("bcls", [1, C_CLS], F32, kind="ExternalInput")
    widx = nc.dram_tensor("widx", [NCH, P, 40], I32, kind="ExternalInput")

    out_t = nc.dram_tensor("out", [1, C_CLS], F32, kind="ExternalOutput")

    zp_sh = nc.dram_tensor("zp_sh", [NLOC, NODE_F32], F32, kind="Internal")
    zp_full = nc.dram_tensor(
        "zp_full", [n_nodes, NODE_F32], F32, kind="Internal",
        addr_space="Shared" if shared_zp else "Local")
    hgp_part = nc.dram_tensor("hgp_part", [1, D], F32, kind="Internal")
    hgp_full = nc.dram_tensor("hgp_full", [NCORES, D], F32, kind="Internal",
                              addr_space="Shared")

    with tile.TileContext(nc) as tc:
        with tc.tile_pool(name="cst", bufs=1) as cst, \
             tc.tile_pool(name="sb", bufs=2) as sb, \
             tc.tile_pool(name="res", bufs=1) as res:

            ident = cst.tile([P, P], F32)
            make_identity(nc, ident[:, :])
            identb = cst.tile([P, P], BF16)
            nc.vector.tensor_copy(out=identb[:, :], in_=ident[:, :])
            identq = cst.tile([P, P], FP8)
            nc.vector.tensor_copy(out=identq[:, :], in_=ident[:, :])
            onesf = cst.tile([P, 1], F32)
            nc.vector.memset(onesf[:, :], 1.0)

            rext = cst.tile([P, 2, 261], BF16)
            nc.sync.dma_start(out=rext[:, 0, :], in_=rext_i[0:P, :])
            nc.sync.dma_start(out=rext[:, 1, :], in_=rext_i[P:D, :])
            nsrc_sb = cst.tile([P, NT], F32)
            nc.sync.dma_start(out=nsrc_sb[:, :], in_=nsrc_i[:, :])
            wcol_rep = cst.tile([P, K], F32)
            nc.sync.dma_start(out=wcol_rep[:, :],
                              in_=wcol_i[0:1, :].to_broadcast([P, K]))
            lw_sb = cst.tile([P, 3, D], BF16)
            nc.vector.memset(lw_sb[:, 0, :], 0.0)
            nc.sync.dma_start(out=lw_sb[0:K - 1, 0, :], in_=lw[0:K - 1, :])
            nc.sync.dma_start(out=lw_sb[:, 1, :], in_=lw[K - 1:K - 1 + P, :])
            nc.sync.dma_start(out=lw_sb[:, 2, :], in_=lw[K - 1 + P:K - 1 + D, :])

            edst_res = res.tile([P, NT], F32)
            ufr = res.tile([P, NT * D], BF16)
            hgs_acc = res.tile([1, D], F32)
            nc.vector.memset(hgs_acc[:, :], 0.0)
            widx_sb = res.tile([P, NCH, 40], I32)
            nc.sync.dma_start(out=widx_sb[:, :, :],
                              in_=widx.rearrange("c p k -> p c k"))

            # ================= phase A: z rows =================
            with tc.tile_pool(name="psa", bufs=2, space="PSUM") as psa:
                zp_wr = []
                for b in range(NT // 4):
                    hT4 = sb.tile([P, 4, 2, P], BF16, tag="hT4")
                    nc.sync.dma_start(out=hT4[:, :, :, :],
                                      in_=hti[:, 4 * b:4 * b + 4, :, :])
                    stg4 = sb.tile([P, 4, NODE_F32], F32, tag="stg4")
                    for i in range(4):
                        t = 4 * b + i
                        zx = psa.tile([P, 261], F32, space="PSUM", tag="zx")
                        nc.tensor.matmul(out=zx[:, :], lhsT=hT4[:, i, 0, :],
                                         rhs=rext[:, 0, :], start=True, stop=False)
                        nc.tensor.matmul(out=zx[:, :], lhsT=hT4[:, i, 1, :],
                                         rhs=rext[:, 1, :], start=False, stop=True)
                        stgq = stg4[:, i, :].bitcast(FP8)
                        nc.scalar.activation(out=stgq[:, 0:D], in_=zx[:, 0:D],
                                             func=AF.Copy)
                        nc.vector.tensor_copy(out=stg4[:, i, SC_ESRC:SC_ESRC + 3],
                                              in_=zx[:, 256:259])
                        nc.vector.tensor_copy(out=edst_res[:, t:t + 1],
                                              in_=zx[:, 259:260])
                        nc.vector.tensor_tensor(out=stg4[:, i, SC_HW:SC_HW + 1],
                                                in0=zx[:, 260:261],
                                                in1=nsrc_sb[:, t:t + 1], op=ALU.mult)
                    w1 = nc.sync.dma_start(
                        out=zp_sh[4 * b * P:(4 * b + 4) * P, :].rearrange(
                            "(a p) f -> p a f", p=P),
                        in_=stg4[:, :, :])
                    zp_wr.append(w1)

            cc_zp = nc.gpsimd.collective_compute(
                "AllGather", ALU.bypass, ins=[zp_sh[:, :]], outs=[zp_full[:, :]],
                replica_groups=rg)
            for w in zp_wr:
                add_dep_helper(cc_zp.ins, w.ins, True, "zp AG after writes")

            # ================= phase C: mailbox =================
            with tc.tile_pool(name="mailp", bufs=4) as mailp, \
                 tc.tile_pool(name="psm", bufs=2, space="PSUM") as psm, \
                 tc.tile_pool(name="psx", bufs=1, space="PSUM") as psx:
                for chh in range(NCH):
                    mail = mailp.tile([P, 40, NODE_F32], F32, tag="mail")
                    for cc_ in range(40):
                        g = nc.gpsimd.indirect_dma_start(
                            out=mail[:, cc_, :],
                            out_offset=None, in_=zp_full[:, :],
                            in_offset=bass.IndirectOffsetOnAxis(
                                ap=widx_sb[:, chh, cc_:cc_ + 1], axis=0))
                        if cc_ == 0:
                            add_dep_helper(g.ins, cc_zp.ins, True,
                                           "gather after zp AG")
                    h_c = sb.tile([P, 4, D], F32, tag="h_c")
                    nc.sync.dma_start(
                        out=h_c[:, :, :],
                        in_=h_in[chh * 512:(chh + 1) * 512, :].rearrange(
                            "(a p) f -> p a f", p=P))

                    esr = mail[:, :, SC_ESRC]
                    zw0 = mail[:, :, SC_ZW0]
                    zw1 = mail[:, :, SC_ZW1]
                    hwg = mail[:, :, SC_HW]

                    ee = sb.tile([P, 40], F32, tag="ee")
                    e3 = ee[:, :].rearrange("p (k j) -> p k j", k=K)
                    ed3 = edst_res[:, chh * 4:(chh + 1) * 4].rearrange(
                        "p (o j) -> p o j", o=1)
                    nc.vector.tensor_tensor(
                        out=e3, in0=esr.rearrange("p (k j) -> p k j", k=K),
                        in1=bc(e3, ed3), op=ALU.add)
                    eesc = sb.tile([P, 40], F32, tag="eesc")
                    nc.vector.tensor_scalar(out=eesc[:, :], in0=ee[:, :],
                                            scalar1=0.01, scalar2=None, op0=ALU.mult)
                    nc.vector.tensor_tensor(out=ee[:, :], in0=ee[:, :],
                                            in1=eesc[:, :], op=ALU.max)
                    emax = sb.tile([P, 4], F32, tag="emax")
                    nc.vector.tensor_reduce(
                        out=emax[:, :], in_=ee[:, :].rearrange("p (k j) -> p j k", k=K),
                        axis=AX.X, op=ALU.max)
                    es = sb.tile([P, 40], F32, tag="es")
                    es3 = es[:, :].rearrange("p (k j) -> p k j", k=K)
                    nc.vector.tensor_tensor(
                        out=es3, in0=e3,
                        in1=bc(es3, emax[:, :].rearrange("p (o j) -> p o j", o=1)),
                        op=ALU.subtract)
                    ex = sb.tile([P, 40], F32, tag="ex")
                    nc.scalar.activation(out=ex[:, :], in_=es[:, :], func=AF.Exp)
                    esum = sb.tile([P, 4], F32, tag="esum")
                    nc.vector.tensor_reduce(
                        out=esum[:, :], in_=ex[:, :].rearrange("p (k j) -> p j k", k=K),
                        axis=AX.X, op=ALU.add)
                    erec = sb.tile([P, 4], F32, tag="erec")
                    nc.vector.reciprocal(out=erec[:, :], in_=esum[:, :])
                    alp = sb.tile([P, 40], F32, tag="alp")
                    al3 = alp[:, :].rearrange("p (k j) -> p k j", k=K)
                    nc.vector.tensor_tensor(
                        out=al3, in0=ex[:, :].rearrange("p (k j) -> p k j", k=K),
                        in1=bc(al3, erec[:, :].rearrange("p (o j) -> p o j", o=1)),
                        op=ALU.mult)

                    bet = sb.tile([P, 40], F32, tag="bet")
                    b3 = bet[:, :].rearrange("p (k j) -> p k j", k=K)
                    nc.vector.tensor_tensor(
                        out=b3, in0=al3,
                        in1=bc(b3, wcol_rep[:, :].rearrange("p (k o) -> p k o", o=1)),
                        op=ALU.mult)

                    agt = sb.tile([P, 4], F32, tag="agt")
                    nc.vector.tensor_reduce(
                        out=agt[:, :], in_=hwg.rearrange("p (k j) -> p j k", k=K),
                        axis=AX.X, op=ALU.add)

                    r0 = sb.tile([P, 40], F32, tag="r0")
                    r1_ = sb.tile([P, 40], F32, tag="r1_")
                    nc.vector.tensor_tensor(out=r0[:, :], in0=alp[:, :], in1=zw0,
                                            op=ALU.mult)
                    nc.vector.tensor_tensor(out=r1_[:, :], in0=alp[:, :], in1=zw1,
                                            op=ALU.mult)
                    rowp = sb.tile([P, 4, 16], F32, tag="rowp")
                    nc.vector.memset(rowp[:, :, K - 1:16], 0.0)
                    nc.vector.tensor_tensor(
                        out=rowp[:, :, 0:K - 1].rearrange("p j k -> p k j"),
                        in0=r0[:, :].rearrange("p (k j) -> p k j", k=K)[:, 0:K - 1, :],
                        in1=r1_[:, :].rearrange("p (k j) -> p k j", k=K)[:, 1:K, :],
                        op=ALU.add)
                    nc.scalar.activation(out=rowp[:, :, 0:K - 1],
                                         in_=rowp[:, :, 0:K - 1], func=AF.Relu,
                                         bias=scal["bias_row"], scale=scal["s_row"])

                    mailq = mail[:, :, :].bitcast(FP8)
                    for jj in range(4):
                        t = chh * 4 + jj
                        colp = psm.tile([P, D], F32, space="PSUM", tag="colp")
                        for k in range(K):
                            cidx = k * 4 + jj
                            dg_ = sb.tile([P, P], FP8, tag="diag")
                            nc.vector.tensor_scalar(
                                out=dg_[:, :], in0=identq[:, :],
                                scalar1=bet[:, cidx:cidx + 1], scalar2=None,
                                op0=ALU.mult)
                            nc.tensor.matmul(
                                out=colp[:, :], lhsT=dg_[:, :],
                                rhs=mailq[:, cidx, 0:D],
                                start=(k == 0), stop=(k == K - 1))
                        colr = sb.tile([P, D], BF16, tag="colr")
                        nc.scalar.activation(out=colr[:, :], in_=colp[:, :],
                                             func=AF.Relu, bias=scal["bias_col"],
                                             scale=scal["s_col"])
                        ctp = psx.tile([P, 2, P], BF16, space="PSUM", tag="ctp")
                        nc.tensor.transpose(out=ctp[:, 0, :], in_=colr[:, 0:P],
                                            identity=identb[:, :])
                        nc.tensor.transpose(out=ctp[:, 1, :], in_=colr[:, P:D],
                                            identity=identb[:, :])
                        colT = sb.tile([P, 2, P], BF16, tag="colT")
                        nc.scalar.copy(out=colT[:, 0, :], in_=ctp[:, 0, :])
                        nc.scalar.copy(out=colT[:, 1, :], in_=ctp[:, 1, :])
                        rtp = psx.tile([16, P], F32, space="PSUM", tag="rtp")
                        nc.tensor.transpose(out=rtp[:, :], in_=rowp[:, jj, :],
                                            identity=ident[:, :])
                        rowT = sb.tile([16, P], BF16, tag="rowT")
                        nc.scalar.copy(out=rowT[:, :], in_=rtp[:, :])
                        ufp = psm.tile([P, D], F32, space="PSUM", tag="ufp")
                        nc.tensor.matmul(out=ufp[:, :], lhsT=rowT[0:K - 1, :],
                                         rhs=lw_sb[0:K - 1, 0, :], start=True, stop=False)
                        nc.tensor.matmul(out=ufp[:, :], lhsT=colT[:, 0, :],
                                         rhs=lw_sb[:, 1, :], start=False, stop=False)
                        nc.tensor.matmul(out=ufp[:, :], lhsT=colT[:, 1, :],
                                         rhs=lw_sb[:, 2, :], start=False, stop=True)
                        ufs = sb.tile([P, D], F32, tag="ufs")
                        nc.vector.tensor_tensor(out=ufs[:, :], in0=ufp[:, :],
                                                in1=h_c[:, jj, :], op=ALU.add)
                        nc.scalar.activation(out=ufr[:, t * D:(t + 1) * D],
                                             in_=ufs[:, :], func=AF.Relu)

                    # group softmax weights (groups of 256 nodes = 2 tiles)
                    # and weighted-mean partials, fused into the chunk
                    eagg = sb.tile([P, 4], BF16, tag="eagg")
                    nc.scalar.activation(out=eagg[:, :], in_=agt[:, :],
                                         func=AF.Exp, bias=scal["b_gc"],
                                         scale=scal["n_dst"])
                    egv = eagg[:, :].rearrange("p (g a) -> p g a", a=2)
                    eg2 = sb.tile([P, 2], F32, tag="eg2")
                    nc.vector.tensor_tensor(out=eg2[:, :], in0=egv[:, :, 0],
                                            in1=egv[:, :, 1], op=ALU.add)
                    zs = psx.tile([1, 2], F32, space="PSUM", tag="zs")
                    nc.tensor.matmul(out=zs[:, :], lhsT=onesf[:, :],
                                     rhs=eg2[:, :], start=True, stop=True)
                    zsum = sb.tile([1, 2], F32, tag="zsum")
                    nc.vector.tensor_copy(out=zsum[:, :], in_=zs[:, :])
                    rz = sb.tile([1, 2], F32, tag="rz")
                    nc.vector.reciprocal(out=rz[:, :], in_=zsum[:, :])
                    for a in range(2):
                        hgq = psx.tile([1, D], F32, space="PSUM", tag="hgq")
                        for j2 in range(2):
                            jj = 2 * a + j2
                            t = chh * 4 + jj
                            nc.tensor.matmul(
                                out=hgq[:, :], lhsT=eagg[:, jj:jj + 1],
                                rhs=ufr[:, t * D:(t + 1) * D],
                                start=(j2 == 0), stop=(j2 == 1))
                        hsc = sb.tile([1, D], F32, tag="hsc")
                        nc.vector.tensor_scalar(
                            out=hsc[:, :], in0=hgq[:, :],
                            scalar1=rz[:, a:a + 1], scalar2=scal["inv_n"],
                            op0=ALU.mult, op1=ALU.mult)
                        nc.vector.tensor_tensor(
                            out=hgs_acc[:, :], in0=hgs_acc[:, :],
                            in1=hsc[:, :], op=ALU.add)

            # ================= phase D: final reduction =================
            with tc.tile_pool(name="pse", bufs=1, space="PSUM") as pse:
                hw3 = nc.sync.dma_start(out=hgp_part[:, :], in_=hgs_acc[:, :])
                cc_hg = nc.gpsimd.collective_compute(
                    "AllGather", ALU.bypass, ins=[hgp_part[:, :]], outs=[hgp_full[:, :]],
                    replica_groups=rg)
                add_dep_helper(cc_hg.ins, hw3.ins, True, "hg AG after write")
                hgf = sb.tile([P, 2, NCORES], F32, tag="hgf")
                for m in range(2):
                    hr = nc.sync.dma_start(
                        out=hgf[:, m, :],
                        in_=hgp_full[:, m * P:(m + 1) * P].rearrange("c p -> p c"))
                    add_dep_helper(hr.ins, cc_hg.ins, True, "hg read after AG")
                hg = sb.tile([P, 2], F32, tag="hg")
                nc.vector.tensor_reduce(
                    out=hg[:, :], in_=hgf[:, :, :], axis=AX.X, op=ALU.add)
                wcls_sb = sb.tile([P, 2, C_CLS], F32, tag="wcls_sb")
                nc.sync.dma_start(out=wcls_sb[:, 0, :], in_=wcls[0:P, :])
                nc.sync.dma_start(out=wcls_sb[:, 1, :], in_=wcls[P:D, :])
                outp = pse.tile([1, C_CLS], F32, space="PSUM", tag="outp")
                for m in range(2):
                    nc.tensor.matmul(out=outp[:, :], lhsT=hg[:, m:m + 1],
                                     rhs=wcls_sb[:, m, :], start=(m == 0), stop=(m == 1))
                bcl = sb.tile([1, C_CLS], F32, tag="bcl")
                nc.sync.dma_start(out=bcl[:, :], in_=bcls[:, :])
                oo = sb.tile([1, C_CLS], F32, tag="oo")
                nc.vector.tensor_tensor(out=oo[:, :], in0=outp[:, :], in1=bcl[:, :],
                                        op=ALU.add)
                nc.sync.dma_start(out=out_t[:, :], in_=oo[:, :])

    return nc


def prep_inputs(h, neighbors, W_fc, a_attn, w_row, b_row, g_row, be_row,
                w_col, b_col, g_col, be_col, localw, W_gc, b_gc, W_cls, b_cls):
    import ml_dtypes
    h = np.asarray(h)
    n_nodes = h.shape[0]
    NLOC = n_nodes // NCORES
    NT = NLOC // P
    NCH = NLOC // 512
    nb = np.asarray(neighbors).astype(np.int64)
    a_attn = np.asarray(a_attn)
    w_row = np.asarray(w_row)

    s_row = float(np.float32(np.asarray(g_row)[0]) / np.sqrt(np.float32(1.0 + EPS)))
    s_col = float(np.float32(np.asarray(g_col)[0]) / np.sqrt(np.float32(1.0 + EPS)))
    scal = dict(
        s_row=s_row,
        bias_row=float(np.float32(np.asarray(b_row)[0]) * np.float32(s_row)
                       + np.float32(np.asarray(be_row)[0])),
        s_col=s_col,
        bias_col=float(np.float32(np.asarray(b_col)[0]) * np.float32(s_col)
                       + np.float32(np.asarray(be_col)[0])),
        n_dst=float(1.0 / np.sqrt(np.float32(K))),
        b_gc=float(np.asarray(b_gc)[0]),
        inv_n=float(np.float32(1.0) / np.float32(n_nodes)),
    )

    deg = np.bincount(nb.ravel(), minlength=n_nodes).astype(np.float64)
    n_src_all = np.where(deg > 0, 1.0 / np.sqrt(np.maximum(deg, 1.0)), 0.0)
    n_src_all = n_src_all.astype(np.float32)

    wfct = np.asarray(W_fc).T.astype(np.float32)
    awp = np.stack([a_attn[:D], w_row[0], w_row[1], a_attn[D:]],
                   axis=1).astype(np.float32)
    v = wfct @ awp                                          # [D, 4]
    rext = np.concatenate(
        [wfct, v, np.asarray(W_gc).astype(np.float32).reshape(D, 1)],
        axis=1).astype(ml_dtypes.bfloat16)                  # [D, 261]

    common = {
        "rext": np.ascontiguousarray(rext),
        "wcol": np.asarray(w_col).astype(np.float32).reshape(1, K),
        "lw": np.asarray(localw).astype(ml_dtypes.bfloat16),
        "wcls": np.asarray(W_cls).astype(np.float32),
        "bcls": np.asarray(b_cls).astype(np.float32).reshape(1, C_CLS),
    }

    in_maps = []
    for c in range(NCORES):
        nbl = nb[c * NLOC:(c + 1) * NLOC]
        widx = np.ascontiguousarray(
            nbl.reshape(NCH, 4, P, K).transpose(0, 2, 3, 1)
            .reshape(NCH, P, 40).astype(np.int32))
        h_loc = h[c * NLOC:(c + 1) * NLOC].astype(np.float32)
        hti = np.ascontiguousarray(
            h_loc.reshape(NT, P, 2, P).transpose(3, 0, 2, 1)
            .astype(ml_dtypes.bfloat16))
        nsrc = np.ascontiguousarray(
            n_src_all[c * NLOC:(c + 1) * NLOC].reshape(NT, P).T)
        m = {
            "h": np.ascontiguousarray(h_loc),
            "hti": hti,
            "nsrc": nsrc,
            "widx": widx,
        }
        m.update(common)
        in_maps.append(m)
    return in_maps, scal, n_nodes


_CACHE = {}


def run(inputs, trace=False):
    _ntff_hook()
    in_maps, scal, n_nodes = prep_inputs(**inputs)
    key = (n_nodes, tuple(sorted(scal.items())))
    if key not in _CACHE:
        nc = build(n_nodes, scal)
        nc.finalize()
        _CACHE[key] = nc
    nc = _CACHE[key]
    return bass_utils.run_bass_kernel_spmd(
        nc, in_maps, core_ids=list(range(NCORES)), trace=trace)


def kernel(**inputs):
    res = run(inputs, trace=False)
    return np.asarray(res.results[0]["out"], dtype=np.float32)


# revision 9
# speedup vs baseline: 1.1856x; 1.0016x over previous
"""Trainium2 Bass kernel for nn_BGAN (GNN message passing), 8 NeuronCores.

Node-sharded SPMD with replicated weights:
  A. z-phase: zx = h_tile @ [W_fc.T | v | W_gc] (bf16 PE, f32 PSUM) where
     v = W_fc.T @ [a_src | w_row0 | w_row1 | a_dst] is host-folded.  Each
     node's 272B table row = z (fp8e4, 256B) + {e_src, zw0, zw1, hw} f32.
     hw = (h@W_gc)*n_src with n_src = rsqrt(out-degree) host-precomputed
     from `neighbors` (index-only preprocessing, like widx).
  B. One AllGather of the 33MiB row table.
  C. mailbox: ONE batched indirect gather per 512-node chunk (5120 rows);
     attention softmax; row conv from gathered scalars; col conv via per-k
     diagonal matmuls accumulated in PSUM; updatefeat matmul.
  D. GraphConv agg -> group softmax weights -> weighted mean folded into
     the final matmul -> AllGather partials -> classifier.

kernel(**inputs): FULL numpy inputs -> FULL [1, C] output.
"""
import sys
import types

import numpy as np

sys.path.insert(0, "/opt/trn_rl_repo")

import concourse.bass as bass
import concourse.bacc as bacc
import concourse.mybir as mybir
import concourse.tile as tile
from concourse import bass_utils
from concourse.bass import broadcast_tensor_aps
from concourse.masks import make_identity
from concourse.tile import add_dep_helper

P = 128
D = 256
K = 10
C_CLS = 40
NCORES = 8
EPS = 1e-5

NODE_F32 = 68            # 272B node row: 256 fp8e4 z + 4 f32 scalars
SC_ESRC = 64
SC_ZW0 = 65
SC_ZW1 = 66
SC_HW = 67

F32 = mybir.dt.float32
BF16 = mybir.dt.bfloat16
FP8 = mybir.dt.float8e4
I32 = mybir.dt.int32
AF = mybir.ActivationFunctionType
ALU = mybir.AluOpType
AX = mybir.AxisListType


def _ntff_hook():
    try:
        import antenv
        from trn_agent_boot.trn_boot import _ntff_profile_via_ctypes
        mod = types.ModuleType("antenv.axon_hooks")
        _state = {"hook": None}
        mod.set_axon_ntff_profile_hook = lambda h: _state.update(hook=h)
        mod.get_axon_ntff_profile_hook = lambda: _state["hook"]
        sys.modules["antenv.axon_hooks"] = mod
        antenv.axon_hooks = mod
        mod.set_axon_ntff_profile_hook(
            _ntff_profile_via_ctypes("/opt/axon/libaxon_pjrt.so"))
    except Exception:
        pass


def bc(a, b):
    """broadcast b against a, return broadcasted b."""
    _, b2 = broadcast_tensor_aps(a, b)
    return b2


def build(n_nodes, scal, shared_zp=True):
    NLOC = n_nodes // NCORES
    NT = NLOC // P
    NCH = NLOC // 512
    NG = NLOC // 256

    nc = bacc.Bacc("TRN2", num_devices=NCORES, dynamic_dma_scratch_size=32768)
    rg = [list(range(NCORES))]

    h_in = nc.dram_tensor("h", [NLOC, D], F32, kind="ExternalInput")
    hti = nc.dram_tensor("hti", [P, NT, 2, P], BF16, kind="ExternalInput")
    rext_i = nc.dram_tensor("rext", [D, 261], BF16, kind="ExternalInput")
    nsrc_i = nc.dram_tensor("nsrc", [P, NT], F32, kind="ExternalInput")
    wcol_i = nc.dram_tensor("wcol", [1, K], F32, kind="ExternalInput")
    lw = nc.dram_tensor("lw", [K - 1 + D, D], BF16, kind="ExternalInput")
    wcls = nc.dram_tensor("wcls", [D, C_CLS], F32, kind="ExternalInput")
    bcls = nc.dram_tensor("bcls", [1, C_CLS], F32, kind="ExternalInput")
    widx = nc.dram_tensor("widx", [NCH, P, 40], I32, kind="ExternalInput")

    out_t = nc.dram_tensor("out", [1, C_CLS], F32, kind="ExternalOutput")

    zp_sh = nc.dram_tensor("zp_sh", [NLOC, NODE_F32], F32, kind="Internal")
    zp_full = nc.dram_tensor(
        "zp_full", [n_nodes, NODE_F32], F32, kind="Internal",
        addr_space="Shared" if shared_zp else "Local")
    hgp_part = nc.dram_tensor("hgp_part", [1, D], F32, kind="Internal")
    hgp_full = nc.dram_tensor("hgp_full", [NCORES, D], F32, kind="Internal",
                              addr_space="Shared")

    with tile.TileContext(nc) as tc:
        with tc.tile_pool(name="cst", bufs=1) as cst, \
             tc.tile_pool(name="sb", bufs=2) as sb, \
             tc.tile_pool(name="res", bufs=1) as res:

            ident = cst.tile([P, P], F32)
            make_identity(nc, ident[:, :])
            identb = cst.tile([P, P], BF16)
            nc.vector.tensor_copy(out=identb[:, :], in_=ident[:, :])
            identq = cst.tile([P, P], FP8)
            nc.vector.tensor_copy(out=identq[:, :], in_=ident[:, :])
            onesf = cst.tile([P, 1], F32)
            nc.vector.memset(onesf[:, :], 1.0)

            rext = cst.tile([P, 2, 261], BF16)
            nc.sync.dma_start(out=rext[:, 0, :], in_=rext_i[0:P, :])
            nc.sync.dma_start(out=rext[:, 1, :], in_=rext_i[P:D, :])
            nsrc_sb = cst.tile([P, NT], F32)
            nc.sync.dma_start(out=nsrc_sb[:, :], in_=nsrc_i[:, :])
            wcol_rep = cst.tile([P, K], F32)
            nc.sync.dma_start(out=wcol_rep[:, :],
                              in_=wcol_i[0:1, :].to_broadcast([P, K]))
            lw_sb = cst.tile([P, 3, D], BF16)
            nc.vector.memset(lw_sb[:, 0, :], 0.0)
            nc.sync.dma_start(out=lw_sb[0:K - 1, 0, :], in_=lw[0:K - 1, :])
            nc.sync.dma_start(out=lw_sb[:, 1, :], in_=lw[K - 1:K - 1 + P, :])
            nc.sync.dma_start(out=lw_sb[:, 2, :], in_=lw[K - 1 + P:K - 1 + D, :])

            edst_res = res.tile([P, NT], F32)
            ufr = res.tile([P, NT * D], BF16)
            hgs_acc = res.tile([1, D], F32)
            nc.vector.memset(hgs_acc[:, :], 0.0)
            widx_sb = res.tile([P, NCH, 40], I32)
            nc.sync.dma_start(out=widx_sb[:, :, :],
                              in_=widx.rearrange("c p k -> p c k"))

            # ================= phase A: z rows =================
            with tc.tile_pool(name="psa", bufs=2, space="PSUM") as psa:
                zp_wr = []
                for b in range(NT // 4):
                    hT4 = sb.tile([P, 4, 2, P], BF16, tag="hT4")
                    nc.sync.dma_start(out=hT4[:, :, :, :],
                                      in_=hti[:, 4 * b:4 * b + 4, :, :])
                    stg4 = sb.tile([P, 4, NODE_F32], F32, tag="stg4")
                    for i in range(4):
                        t = 4 * b + i
                        zx = psa.tile([P, 261], F32, space="PSUM", tag="zx")
                        nc.tensor.matmul(out=zx[:, :], lhsT=hT4[:, i, 0, :],
                                         rhs=rext[:, 0, :], start=True, stop=False)
                        nc.tensor.matmul(out=zx[:, :], lhsT=hT4[:, i, 1, :],
                                         rhs=rext[:, 1, :], start=False, stop=True)
                        stgq = stg4[:, i, :].bitcast(FP8)
                        nc.scalar.activation(out=stgq[:, 0:D], in_=zx[:, 0:D],
                                             func=AF.Copy)
                        nc.vector.tensor_copy(out=stg4[:, i, SC_ESRC:SC_ESRC + 3],
                                              in_=zx[:, 256:259])
                        nc.vector.tensor_copy(out=edst_res[:, t:t + 1],
                                              in_=zx[:, 259:260])
                        nc.vector.tensor_tensor(out=stg4[:, i, SC_HW:SC_HW + 1],
                                                in0=zx[:, 260:261],
                                                in1=nsrc_sb[:, t:t + 1], op=ALU.mult)
                    w1 = nc.sync.dma_start(
                        out=zp_sh[4 * b * P:(4 * b + 4) * P, :].rearrange(
                            "(a p) f -> p a f", p=P),
                        in_=stg4[:, :, :])
                    zp_wr.append(w1)

            half = NLOC // 2
            ngrp = len(zp_wr)
            cc_zp1 = nc.gpsimd.collective_compute(
                "AllGather", ALU.bypass, ins=[zp_sh[0:half, :]],
                outs=[zp_full[0:n_nodes // 2, :]], replica_groups=rg)
            cc_zp2 = nc.gpsimd.collective_compute(
                "AllGather", ALU.bypass, ins=[zp_sh[half:NLOC, :]],
                outs=[zp_full[n_nodes // 2:n_nodes, :]], replica_groups=rg)
            for gi, w in enumerate(zp_wr):
                first_node = gi * (NLOC // ngrp)
                if first_node < half:
                    add_dep_helper(cc_zp1.ins, w.ins, True, "zp AG1 after writes")
                add_dep_helper(cc_zp2.ins, w.ins, True, "zp AG2 after writes")

            # ================= phase C: mailbox =================
            with tc.tile_pool(name="mailp", bufs=4) as mailp, \
                 tc.tile_pool(name="psm", bufs=2, space="PSUM") as psm, \
                 tc.tile_pool(name="psx", bufs=1, space="PSUM") as psx:
                for chh in range(NCH):
                    mail = mailp.tile([P, 40, NODE_F32], F32, tag="mail")
                    for cc_ in range(40):
                        g = nc.gpsimd.indirect_dma_start(
                            out=mail[:, cc_, :],
                            out_offset=None, in_=zp_full[:, :],
                            in_offset=bass.IndirectOffsetOnAxis(
                                ap=widx_sb[:, chh, cc_:cc_ + 1], axis=0))
                        if cc_ == 0:
                            add_dep_helper(g.ins, cc_zp1.ins, True,
                                           "gather after zp AG1")
                            add_dep_helper(g.ins, cc_zp2.ins, True,
                                           "gather after zp AG2")
                    h_c = sb.tile([P, 4, D], F32, tag="h_c")
                    nc.sync.dma_start(
                        out=h_c[:, :, :],
                        in_=h_in[chh * 512:(chh + 1) * 512, :].rearrange(
                            "(a p) f -> p a f", p=P))

                    esr = mail[:, :, SC_ESRC]
                    zw0 = mail[:, :, SC_ZW0]
                    zw1 = mail[:, :, SC_ZW1]
                    hwg = mail[:, :, SC_HW]

                    ee = sb.tile([P, 40], F32, tag="ee")
                    e3 = ee[:, :].rearrange("p (k j) -> p k j", k=K)
                    ed3 = edst_res[:, chh * 4:(chh + 1) * 4].rearrange(
                        "p (o j) -> p o j", o=1)
                    nc.vector.tensor_tensor(
                        out=e3, in0=esr.rearrange("p (k j) -> p k j", k=K),
                        in1=bc(e3, ed3), op=ALU.add)
                    eesc = sb.tile([P, 40], F32, tag="eesc")
                    nc.vector.tensor_scalar(out=eesc[:, :], in0=ee[:, :],
                                            scalar1=0.01, scalar2=None, op0=ALU.mult)
                    nc.vector.tensor_tensor(out=ee[:, :], in0=ee[:, :],
                                            in1=eesc[:, :], op=ALU.max)
                    emax = sb.tile([P, 4], F32, tag="emax")
                    nc.vector.tensor_reduce(
                        out=emax[:, :], in_=ee[:, :].rearrange("p (k j) -> p j k", k=K),
                        axis=AX.X, op=ALU.max)
                    es = sb.tile([P, 40], F32, tag="es")
                    es3 = es[:, :].rearrange("p (k j) -> p k j", k=K)
                    nc.vector.tensor_tensor(
                        out=es3, in0=e3,
                        in1=bc(es3, emax[:, :].rearrange("p (o j) -> p o j", o=1)),
                        op=ALU.subtract)
                    ex = sb.tile([P, 40], F32, tag="ex")
                    nc.scalar.activation(out=ex[:, :], in_=es[:, :], func=AF.Exp)
                    esum = sb.tile([P, 4], F32, tag="esum")
                    nc.vector.tensor_reduce(
                        out=esum[:, :], in_=ex[:, :].rearrange("p (k j) -> p j k", k=K),
                        axis=AX.X, op=ALU.add)
                    erec = sb.tile([P, 4], F32, tag="erec")
                    nc.vector.reciprocal(out=erec[:, :], in_=esum[:, :])
                    alp = sb.tile([P, 40], F32, tag="alp")
                    al3 = alp[:, :].rearrange("p (k j) -> p k j", k=K)
                    nc.vector.tensor_tensor(
                        out=al3, in0=ex[:, :].rearrange("p (k j) -> p k j", k=K),
                        in1=bc(al3, erec[:, :].rearrange("p (o j) -> p o j", o=1)),
                        op=ALU.mult)

                    bet = sb.tile([P, 40], F32, tag="bet")
                    b3 = bet[:, :].rearrange("p (k j) -> p k j", k=K)
                    nc.vector.tensor_tensor(
                        out=b3, in0=al3,
                        in1=bc(b3, wcol_rep[:, :].rearrange("p (k o) -> p k o", o=1)),
                        op=ALU.mult)

                    agt = sb.tile([P, 4], F32, tag="agt")
                    nc.vector.tensor_reduce(
                        out=agt[:, :], in_=hwg.rearrange("p (k j) -> p j k", k=K),
                        axis=AX.X, op=ALU.add)

                    r0 = sb.tile([P, 40], F32, tag="r0")
                    r1_ = sb.tile([P, 40], F32, tag="r1_")
                    nc.vector.tensor_tensor(out=r0[:, :], in0=alp[:, :], in1=zw0,
                                            op=ALU.mult)
                    nc.vector.tensor_tensor(out=r1_[:, :], in0=alp[:, :], in1=zw1,
                                            op=ALU.mult)
                    rowp = sb.tile([P, 4, 16], F32, tag="rowp")
                    nc.vector.memset(rowp[:, :, K - 1:16], 0.0)
                    nc.vector.tensor_tensor(
                        out=rowp[:, :, 0:K - 1].rearrange("p j k -> p k j"),
                        in0=r0[:, :].rearrange("p (k j) -> p k j", k=K)[:, 0:K - 1, :],
                        in1=r1_[:, :].rearrange("p (k j) -> p k j", k=K)[:, 1:K, :],
                        op=ALU.add)
                    nc.scalar.activation(out=rowp[:, :, 0:K - 1],
                                         in_=rowp[:, :, 0:K - 1], func=AF.Relu,
                                         bias=scal["bias_row"], scale=scal["s_row"])

                    mailq = mail[:, :, :].bitcast(FP8)
                    for jj in range(4):
                        t = chh * 4 + jj
                        colp = psm.tile([P, D], F32, space="PSUM", tag="colp")
                        for k in range(K):
                            cidx = k * 4 + jj
                            dg_ = sb.tile([P, P], FP8, tag="diag")
                            nc.vector.tensor_scalar(
                                out=dg_[:, :], in0=identq[:, :],
                                scalar1=bet[:, cidx:cidx + 1], scalar2=None,
                                op0=ALU.mult)
                            nc.tensor.matmul(
                                out=colp[:, :], lhsT=dg_[:, :],
                                rhs=mailq[:, cidx, 0:D],
                                start=(k == 0), stop=(k == K - 1))
                        colr = sb.tile([P, D], BF16, tag="colr")
                        nc.scalar.activation(out=colr[:, :], in_=colp[:, :],
                                             func=AF.Relu, bias=scal["bias_col"],
                                             scale=scal["s_col"])
                        ctp = psx.tile([P, 2, P], BF16, space="PSUM", tag="ctp")
                        nc.tensor.transpose(out=ctp[:, 0, :], in_=colr[:, 0:P],
                                            identity=identb[:, :])
                        nc.tensor.transpose(out=ctp[:, 1, :], in_=colr[:, P:D],
                                            identity=identb[:, :])
                        colT = sb.tile([P, 2, P], BF16, tag="colT")
                        nc.scalar.copy(out=colT[:, 0, :], in_=ctp[:, 0, :])
                        nc.scalar.copy(out=colT[:, 1, :], in_=ctp[:, 1, :])
                        rtp = psx.tile([16, P], F32, space="PSUM", tag="rtp")
                        nc.tensor.transpose(out=rtp[:, :], in_=rowp[:, jj, :],
                                            identity=ident[:, :])
                        rowT = sb.tile([16, P], BF16, tag="rowT")
                        nc.scalar.copy(out=rowT[:, :], in_=rtp[:, :])
                        ufp = psm.tile([P, D], F32, space="PSUM", tag="ufp")
                        nc.tensor.matmul(out=ufp[:, :], lhsT=rowT[0:K - 1, :],
                                         rhs=lw_sb[0:K - 1, 0, :], start=True, stop=False)
                        nc.tensor.matmul(out=ufp[:, :], lhsT=colT[:, 0, :],
                                         rhs=lw_sb[:, 1, :], start=False, stop=False)
                        nc.tensor.matmul(out=ufp[:, :], lhsT=colT[:, 1, :],
                                         rhs=lw_sb[:, 2, :], start=False, stop=True)
                        ufs = sb.tile([P, D], F32, tag="ufs")
                        nc.vector.tensor_tensor(out=ufs[:, :], in0=ufp[:, :],
                                                in1=h_c[:, jj, :], op=ALU.add)
                        nc.scalar.activation(out=ufr[:, t * D:(t + 1) * D],
                                             in_=ufs[:, :], func=AF.Relu)

                    # group softmax weights (groups of 256 nodes = 2 tiles)
                    # and weighted-mean partials, fused into the chunk
                    eagg = sb.tile([P, 4], BF16, tag="eagg")
                    nc.scalar.activation(out=eagg[:, :], in_=agt[:, :],
                                         func=AF.Exp, bias=scal["b_gc"],
                                         scale=scal["n_dst"])
                    egv = eagg[:, :].rearrange("p (g a) -> p g a", a=2)
                    eg2 = sb.tile([P, 2], F32, tag="eg2")
                    nc.vector.tensor_tensor(out=eg2[:, :], in0=egv[:, :, 0],
                                            in1=egv[:, :, 1], op=ALU.add)
                    zs = psx.tile([1, 2], F32, space="PSUM", tag="zs")
                    nc.tensor.matmul(out=zs[:, :], lhsT=onesf[:, :],
                                     rhs=eg2[:, :], start=True, stop=True)
                    zsum = sb.tile([1, 2], F32, tag="zsum")
                    nc.vector.tensor_copy(out=zsum[:, :], in_=zs[:, :])
                    rz = sb.tile([1, 2], F32, tag="rz")
                    nc.vector.reciprocal(out=rz[:, :], in_=zsum[:, :])
                    for a in range(2):
                        hgq = psx.tile([1, D], F32, space="PSUM", tag="hgq")
                        for j2 in range(2):
                            jj = 2 * a + j2
                            t = chh * 4 + jj
                            nc.tensor.matmul(
                                out=hgq[:, :], lhsT=eagg[:, jj:jj + 1],
                                rhs=ufr[:, t * D:(t + 1) * D],
                                start=(j2 == 0), stop=(j2 == 1))
                        hsc = sb.tile([1, D], F32, tag="hsc")
                        nc.vector.tensor_scalar(
                            out=hsc[:, :], in0=hgq[:, :],
                            scalar1=rz[:, a:a + 1], scalar2=scal["inv_n"],
                            op0=ALU.mult, op1=ALU.mult)
                        nc.vector.tensor_tensor(
                            out=hgs_acc[:, :], in0=hgs_acc[:, :],
                            in1=hsc[:, :], op=ALU.add)

            # ================= phase D: final reduction =================
            with tc.tile_pool(name="pse", bufs=1, space="PSUM") as pse:
                hw3 = nc.sync.dma_start(out=hgp_part[:, :], in_=hgs_acc[:, :])
                cc_hg = nc.gpsimd.collective_compute(
                    "AllGather", ALU.bypass, ins=[hgp_part[:, :]], outs=[hgp_full[:, :]],
                    replica_groups=rg)
                add_dep_helper(cc_hg.ins, hw3.ins, True, "hg AG after write")
                hgf = sb.tile([P, 2, NCORES], F32, tag="hgf")
                for m in range(2):
                    hr = nc.sync.dma_start(
                        out=hgf[:, m, :],
                        in_=hgp_full[:, m * P:(m + 1) * P].rearrange("c p -> p c"))
                    add_dep_helper(hr.ins, cc_hg.ins, True, "hg read after AG")
                hg = sb.tile([P, 2], F32, tag="hg")
                nc.vector.tensor_reduce(
                    out=hg[:, :], in_=hgf[:, :, :], axis=AX.X, op=ALU.add)
                wcls_sb = sb.tile([P, 2, C_CLS], F32, tag="wcls_sb")
                nc.sync.dma_start(out=wcls_sb[:, 0, :], in_=wcls[0:P, :])
                nc.sync.dma_start(out=wcls_sb[:, 1, :], in_=wcls[P:D, :])
                outp = pse.tile([1, C_CLS], F32, space="PSUM", tag="outp")
                for m in range(2):
                    nc.tensor.matmul(out=outp[:, :], lhsT=hg[:, m:m + 1],
                                     rhs=wcls_sb[:, m, :], start=(m == 0), stop=(m == 1))
                bcl = sb.tile([1, C_CLS], F32, tag="bcl")
                nc.sync.dma_start(out=bcl[:, :], in_=bcls[:, :])
                oo = sb.tile([1, C_CLS], F32, tag="oo")
                nc.vector.tensor_tensor(out=oo[:, :], in0=outp[:, :], in1=bcl[:, :],
                                        op=ALU.add)
                nc.sync.dma_start(out=out_t[:, :], in_=oo[:, :])

    return nc


def prep_inputs(h, neighbors, W_fc, a_attn, w_row, b_row, g_row, be_row,
                w_col, b_col, g_col, be_col, localw, W_gc, b_gc, W_cls, b_cls):
    import ml_dtypes
    h = np.asarray(h)
    n_nodes = h.shape[0]
    NLOC = n_nodes // NCORES
    NT = NLOC // P
    NCH = NLOC // 512
    nb = np.asarray(neighbors).astype(np.int64)
    a_attn = np.asarray(a_attn)
    w_row = np.asarray(w_row)

    s_row = float(np.float32(np.asarray(g_row)[0]) / np.sqrt(np.float32(1.0 + EPS)))
    s_col = float(np.float32(np.asarray(g_col)[0]) / np.sqrt(np.float32(1.0 + EPS)))
    scal = dict(
        s_row=s_row,
        bias_row=float(np.float32(np.asarray(b_row)[0]) * np.float32(s_row)
                       + np.float32(np.asarray(be_row)[0])),
        s_col=s_col,
        bias_col=float(np.float32(np.asarray(b_col)[0]) * np.float32(s_col)
                       + np.float32(np.asarray(be_col)[0])),
        n_dst=float(1.0 / np.sqrt(np.float32(K))),
        b_gc=float(np.asarray(b_gc)[0]),
        inv_n=float(np.float32(1.0) / np.float32(n_nodes)),
    )

    deg = np.bincount(nb.ravel(), minlength=n_nodes).astype(np.float64)
    n_src_all = np.where(deg > 0, 1.0 / np.sqrt(np.maximum(deg, 1.0)), 0.0)
    n_src_all = n_src_all.astype(np.float32)

    wfct = np.asarray(W_fc).T.astype(np.float32)
    awp = np.stack([a_attn[:D], w_row[0], w_row[1], a_attn[D:]],
                   axis=1).astype(np.float32)
    v = wfct @ awp                                          # [D, 4]
    rext = np.concatenate(
        [wfct, v, np.asarray(W_gc).astype(np.float32).reshape(D, 1)],
        axis=1).astype(ml_dtypes.bfloat16)                  # [D, 261]

    common = {
        "rext": np.ascontiguousarray(rext),
        "wcol": np.asarray(w_col).astype(np.float32).reshape(1, K),
        "lw": np.asarray(localw).astype(ml_dtypes.bfloat16),
        "wcls": np.asarray(W_cls).astype(np.float32),
        "bcls": np.asarray(b_cls).astype(np.float32).reshape(1, C_CLS),
    }

    NL2 = NLOC // 2
    cc = nb // NLOC
    off = nb % NLOC
    nbr = np.where(off < NL2, cc * NL2 + off,
                   n_nodes // 2 + cc * NL2 + (off - NL2))
    in_maps = []
    for c in range(NCORES):
        nbl = nbr[c * NLOC:(c + 1) * NLOC]
        widx = np.ascontiguousarray(
            nbl.reshape(NCH, 4, P, K).transpose(0, 2, 3, 1)
            .reshape(NCH, P, 40).astype(np.int32))
        h_loc = h[c * NLOC:(c + 1) * NLOC].astype(np.float32)
        hti = np.ascontiguousarray(
            h_loc.reshape(NT, P, 2, P).transpose(3, 0, 2, 1)
            .astype(ml_dtypes.bfloat16))
        nsrc = np.ascontiguousarray(
            n_src_all[c * NLOC:(c + 1) * NLOC].reshape(NT, P).T)
        m = {
            "h": np.ascontiguousarray(h_loc),
            "hti": hti,
            "nsrc": nsrc,
            "widx": widx,
        }
        m.update(common)
        in_maps.append(m)
    return in_maps, scal, n_nodes


_CACHE = {}


def run(inputs, trace=False):
    _ntff_hook()
    in_maps, scal, n_nodes = prep_inputs(**inputs)
    key = (n_nodes, tuple(sorted(scal.items())))
    if key not in _CACHE:
        nc = build(n_nodes, scal)
        nc.finalize()
        _CACHE[key] = nc
    nc = _CACHE[key]
    return bass_utils.run_bass_kernel_spmd(
        nc, in_maps, core_ids=list(range(NCORES)), trace=trace)


def kernel(**inputs):
    res = run(inputs, trace=False)
    return np.asarray(res.results[0]["out"], dtype=np.float32)
